# revision 12
# baseline (speedup 1.0000x reference)
"""Trainium2 Bass kernel for the delta-rule memory recurrence (DeltaNet-style).

Full-input contract: kernel(memory, key, value) -> final memory, all np.ndarray,
shapes (16,256,256), (16,4096,256), (16,4096,256) -> (16,256,256) float32.

Strategy: pure data-parallel over batch (2 batches per NeuronCore x 8 cores).
Per batch the sequential recurrence

    kn   = k_t / ||k_t||
    M   <- M - (1.1 * M kn - 0.1 * v_t) kn^T

is reformulated chunkwise (C=128 steps per chunk) via the WY / UT transform:

    A  = Kn Kn^T                      (C x C Gram of normalized keys)
    L  = 1.1 * strict_lower(A)
    Tinv = (I + L)^{-1}               (unit lower triangular inverse)
    H  = Tinv @ (-1.1 * Kn Mt + 0.1 * V)
    Mt <- Mt + Kn^T H                 (Mt = M^T state, (DK, DV))

(I+L)^{-1} is computed exactly with the nilpotent factorization
(I-L)(I+L^2)(I+L^4)(I+L^8)  [L^16 and beyond are numerically zero here].
Inversion machinery runs in fp16 matmuls (full PE rate, 10-bit mantissa),
state-path matmuls run as float32r (full rate at N>=256).
"""

import numpy as np

import concourse.bass as bass
import concourse.mybir as mybir
import concourse.tile as tile
from concourse.bass import ts
from concourse.bass_utils import run_bass_kernel_spmd
from concourse.masks import make_identity

F32 = mybir.dt.float32
F32R = mybir.dt.float32r
F16 = mybir.dt.float16
AOP = mybir.AluOpType
AFT = mybir.ActivationFunctionType

B, S, DK, DV = 16, 4096, 256, 256
NCORES = 8
BLOC = B // NCORES          # batches per core
C = 128                     # chunk length
LR = 0.1
AC = 1.0 + LR               # 1.1
NLEV = 3                    # squaring levels: (I-L)(I+L^2)(I+L^4)(I+L^8)


def _split_waits(nc, max_waits=1):
    """walrus codegen on this toolchain encodes at most one semaphore wait per
    instruction; hoist excess waits onto same-engine NoOps placed just before."""
    n_split = 0
    for f in nc.m.functions:
        for bb in f.blocks:
            insts = bb.instructions
            out = []
            for inst in insts:
                si = getattr(inst, "sync_info", None)
                w = list(si.on_wait) if (si and si.on_wait) else []
                k = 0
                while len(w) > max_waits:
                    head, w = w[:max_waits], w[max_waits:]
                    out.append(mybir.InstNoOp(
                        name=f"{inst.name}-wsplit{k}",
                        engine=inst.engine,
                        sync_info=mybir.SyncInfo(on_wait=head, on_update=[]),
                    ))
                    n_split += 1
                    k += 1
                if k:
                    inst.sync_info = mybir.SyncInfo(
                        on_wait=w, on_update=list(si.on_update or [])
                    )
                out.append(inst)
            bb.instructions = out
    return n_split


def build_nc(s_loc=S, state_mm_dtype=F32R, split=True):
    nch = s_loc // C
    nc = bass.Bass()
    memT = nc.declare_dram_parameter("memT", [BLOC, DK, DV], F32, isOutput=False)
    key_d = nc.declare_dram_parameter("key", [BLOC, s_loc, DK], F32, isOutput=False)
    val_d = nc.declare_dram_parameter("value", [BLOC, s_loc, DV], F32, isOutput=False)
    outT = nc.declare_dram_parameter("outT", [BLOC, DK, DV], F32, isOutput=True)

    SMM = state_mm_dtype  # state-path matmul tiles (float32r: full-rate fp32-ish mm)

    with tile.TileContext(nc) as tc:
        with (
            tc.tile_pool(name="consts", bufs=1) as consts,
            tc.tile_pool(name="kv", bufs=3) as kv,
            tc.tile_pool(name="norm", bufs=3) as normp,
            tc.tile_pool(name="kt", bufs=3) as ktp,
            tc.tile_pool(name="inv", bufs=2) as invp,
            tc.tile_pool(name="state", bufs=2) as statep,
            tc.tile_pool(name="mt", bufs=2) as mtp,
            tc.tile_pool(name="ps_inv", bufs=4, space="PSUM") as ps_inv,
            tc.tile_pool(name="ps_state", bufs=2, space="PSUM") as ps_state,
            tc.tile_pool(name="ps_upd", bufs=2, space="PSUM") as ps_upd,
        ):
            ident32 = consts.tile([128, 128], F32, tag="ident32")
            make_identity(nc, ident32)
            ident16 = consts.tile([128, 128], F16, tag="ident16")
            make_identity(nc, ident16)
            # paired identity (both halves) for G0 = I + LTn
            i2_16 = consts.tile([128, 2, 128], F16, tag="i2_16")
            nc.gpsimd.memset(i2_16, 0.0)
            nc.gpsimd.affine_select(
                out=i2_16, in_=i2_16, compare_op=AOP.not_equal, fill=1.0,
                base=0, pattern=[[0, 2], [-1, 128]], channel_multiplier=1,
            )

            # initial state Mt (= M^T) per batch, laid out [p, ktile, v]
            mt = []
            for b in range(BLOC):
                t0 = mtp.tile([128, 2, DV], F32, tag=f"mt0f{b}")
                nc.sync.dma_start(
                    out=t0, in_=memT[b].rearrange("(j p) v -> p j v", p=128)
                )
                t = mtp.tile([128, 2, DV], SMM, tag=f"mt{b}")
                nc.vector.tensor_copy(t, t0)
                mt.append(t)

            for c in range(nch):
                Kt, Vt, Kn, V01h, rn = [], [], [], [], []
                for b in range(BLOC):
                    k = kv.tile([128, DK], F32, tag=f"k{b}")
                    nc.sync.dma_start(out=k, in_=key_d[b, c * C:(c + 1) * C, :])
                    v = kv.tile([128, DV], F32, tag=f"v{b}")
                    nc.sync.dma_start(out=v, in_=val_d[b, c * C:(c + 1) * C, :])
                    Kt.append(k)
                    Vt.append(v)
                for b in range(BLOC):
                    scr = normp.tile([128, DK], F32, tag="scr")
                    ssq = normp.tile([128, 1], F32, tag=f"ssq{b}")
                    nc.scalar.activation(out=scr, in_=Kt[b], func=AFT.Square,
                                         accum_out=ssq)
                    nrm = normp.tile([128, 1], F32, tag=f"nrm{b}")
                    nc.scalar.activation(nrm, ssq, AFT.Sqrt)
                    r = normp.tile([128, 1], F32, tag=f"rn{b}")
                    nc.vector.reciprocal(r, nrm)
                    rn.append(r)
                    kn = normp.tile([128, DK], SMM, tag=f"kn{b}")
                    nc.vector.tensor_scalar_mul(kn, Kt[b], r)
                    Kn.append(kn)
                    vh = normp.tile([128, DV], F16, tag=f"v01h{b}")
                    nc.gpsimd.tensor_scalar_mul(vh, Vt[b], LR)
                    V01h.append(vh)

                # transposes of Kn: KnTs (f32, for state path) + KnTh (f16, for A)
                KnTs = [[None] * 2 for _ in range(BLOC)]
                KnTh = [[None] * 2 for _ in range(BLOC)]
                for b in range(BLOC):
                    for j in range(2):
                        tp = ps_inv.tile([128, 128], F32, tag="inv")
                        nc.tensor.transpose(
                            tp, Kn[b][:, ts(j, 128)].bitcast(F32), ident32)
                        s32 = ktp.tile([128, 128], SMM, tag=f"knts{b}{j}")
                        nc.scalar.copy(s32, tp)
                        s16 = ktp.tile([128, 128], F16, tag=f"knth{b}{j}")
                        nc.vector.tensor_copy(s16, tp)
                        KnTs[b][j] = s32
                        KnTh[b][j] = s16

                # A = Kn Kn^T (both batches share a [128, 2, 128] psum tile)
                a_ps = ps_inv.tile([128, 2, 128], F32, tag="inv")
                for b in range(BLOC):
                    for j in range(2):
                        nc.tensor.matmul(
                            a_ps[:, b, :], KnTh[b][j], KnTh[b][j],
                            start=(j == 0), stop=(j == 1),
                        )
                a_neg = invp.tile([128, 2, 128], F16, tag="a_neg")
                nc.scalar.mul(a_neg, a_ps, -AC)
                # Ln = -L = strict_lower(a_neg); LTn = -L^T = strict_upper(a_neg)
                ln = invp.tile([128, 2, 128], F16, tag="ln")
                nc.gpsimd.affine_select(
                    out=ln, in_=a_neg, compare_op=AOP.is_gt, fill=0.0,
                    base=0, pattern=[[0, 2], [-1, 128]], channel_multiplier=1,
                )
                ltn = invp.tile([128, 2, 128], F16, tag="ltn")
                nc.gpsimd.affine_select(
                    out=ltn, in_=a_neg, compare_op=AOP.is_gt, fill=0.0,
                    base=0, pattern=[[0, 2], [1, 128]], channel_multiplier=-1,
                )

                # power chain: L2 = LTn^T@Ln, LT2 = Ln^T@LTn, L4, LT4, L8
                def pair_mm(lhsT, rhs, tag, engine_copy):
                    ps = ps_inv.tile([128, 2, 128], F32, tag="inv")
                    for b in range(BLOC):
                        nc.tensor.matmul(ps[:, b, :], lhsT[:, b, :], rhs[:, b, :])
                    sb = invp.tile([128, 2, 128], F16, tag=tag)
                    if engine_copy == "v":
                        nc.vector.tensor_copy(sb, ps)
                    else:
                        nc.scalar.copy(sb, ps)
                    return sb

                l2 = pair_mm(ltn, ln, "l2", "v")
                lt2 = pair_mm(ln, ltn, "lt2", "s")
                l4 = pair_mm(lt2, l2, "l4", "v")
                lt4 = pair_mm(l2, lt2, "lt4", "s")
                l8 = pair_mm(lt4, l4, "l8", "v")

                # G chain: G0 = I + LTn; G <- (I + LT^{2^i}) G via psum inject
                g = invp.tile([128, 2, 128], F16, tag="g0")
                nc.vector.tensor_add(g, i2_16, ltn)
                for i, lp in enumerate((l2, l4, l8)):
                    gp = ps_inv.tile([128, 2, 128], F32, tag="inv")
                    for b in range(BLOC):
                        nc.tensor.matmul(gp[:, b, :], lp[:, b, :], g[:, b, :],
                                         start=True, stop=False)
                        nc.tensor.matmul(gp[:, b, :], ident16, g[:, b, :],
                                         start=False, stop=True)
                    gn = invp.tile([128, 2, 128], F16, tag=f"g{i + 1}")
                    if i % 2 == 0:
                        nc.scalar.copy(gn, gp)
                    else:
                        nc.vector.tensor_copy(gn, gp)
                    g = gn

                # state path per batch
                for b in range(BLOC):
                    y_ps = ps_state.tile([128, DV], F32, tag="st")
                    for j in range(2):
                        nc.tensor.matmul(
                            y_ps, KnTs[b][j], mt[b][:, j, :],
                            start=(j == 0), stop=(j == 1),
                        )
                    rh = statep.tile([128, DV], F16, tag=f"rh{b}")
                    # R = (-1.1 * Kn Mt) + 0.1 V   (fp16 for the Tinv apply)
                    nc.vector.scalar_tensor_tensor(
                        out=rh, in0=y_ps, scalar=-AC, in1=V01h[b],
                        op0=AOP.mult, op1=AOP.add,
                    )
                    h_ps = ps_state.tile([128, DV], F32, tag="st")
                    nc.tensor.matmul(h_ps, g[:, b, :], rh)
                    h_sb = statep.tile([128, DV], SMM, tag=f"hs{b}")
                    nc.scalar.copy(h_sb, h_ps)
                    u_ps = ps_upd.tile([128, 2, DV], F32, tag="u")
                    for j in range(2):
                        nc.tensor.matmul(
                            u_ps[:, j, :], Kn[b][:, ts(j, 128)], h_sb,
                        )
                    mt_new = mtp.tile([128, 2, DV], SMM, tag=f"mt{b}")
                    nc.vector.tensor_add(mt_new, mt[b], u_ps)
                    mt[b] = mt_new

            for b in range(BLOC):
                nc.sync.dma_start(
                    out=outT[b].rearrange("(j p) v -> p j v", p=128),
                    in_=mt[b].bitcast(F32),
                )
    if split:
        _split_waits(nc)
    return nc


_NC_CACHE = {}

# test-harness hooks (the grading harness just calls kernel())
TRACE = False
LAST_RESULT = None


def _get_nc(s_loc=S):
    if s_loc not in _NC_CACHE:
        _NC_CACHE[s_loc] = build_nc(s_loc)
    return _NC_CACHE[s_loc]


def kernel(memory, key, value):
    global LAST_RESULT
    memory = np.ascontiguousarray(np.asarray(memory), dtype=np.float32)
    key = np.ascontiguousarray(np.asarray(key), dtype=np.float32)
    value = np.ascontiguousarray(np.asarray(value), dtype=np.float32)
    s_loc = key.shape[1]
    nc = _get_nc(s_loc)
    memT = np.ascontiguousarray(memory.transpose(0, 2, 1))
    in_maps = []
    for i in range(NCORES):
        sl = slice(i * BLOC, (i + 1) * BLOC)
        in_maps.append({
            "memT": memT[sl],
            "key": np.ascontiguousarray(key[sl]),
            "value": np.ascontiguousarray(value[sl]),
        })
    res = run_bass_kernel_spmd(nc, in_maps, list(range(NCORES)), trace=TRACE)
    LAST_RESULT = res
    outs = [res.results[i]["outT"] for i in range(NCORES)]
    out = np.concatenate(outs, axis=0)          # (16, DK, DV) = M^T
    return np.ascontiguousarray(out.transpose(0, 2, 1))
